# revision 16
# baseline (speedup 1.0000x reference)
"""BotRGCN on 8 Trainium2 NeuronCores (Bass/Tile, SPMD).

Strategy (per sharding hint): nodes row-sharded across 8 cores, edges
partitioned by destination node. Per RGCN layer, each core computes the
per-relation transformed features h_r = x @ W_r for its node shard
(node-major, f16). The h table is exchanged with FOUR pipelined
AllGathers (one per tile-aligned quarter of the node shard) so that
message aggregation for quarter g overlaps the collective for quarter
g+1. Each core aggregates messages for its destination shard with
prepared indexed DMA row-gathers (descriptor emission decoupled from
the h dependency via prepare_only/trigger_dma) followed by one-hot
S-matrix matmuls on the tensor engine that perform the segment-mean
(1/cnt folded into S). S blocks are precomputed on the host and
streamed from HBM (f16). Aggregation runs quarter-outer into an SBUF
f32 accumulator; root terms are computed directly into the accumulator
during the AllGathers. All dense matmuls run in f16 (f32 PSUM).

Self-contained: hardcodes problem shapes; the host side shards inputs,
builds gather indices + S blocks, compiles one SPMD Bass program and
runs it on cores 0-7.
"""
import sys

import numpy as np

for _p in ("/opt/trn_rl_repo",):
    if _p not in sys.path:
        sys.path.insert(0, _p)

import concourse.bacc as bacc
import concourse.mybir as mybir
from concourse import tile

dt = mybir.dt

NCORES = 8
SLOPE = 0.01
DEFAULT_VARIANT = "sp0q4"
NQUART = 4


def _ceil_to(x, m):
    return ((x + m - 1) // m) * m


class Meta:
    pass


def prepare(inputs, ncores=NCORES, node_chunk=448):
    """Shard inputs, build gather indices + host-precomputed S blocks."""
    m = Meta()
    N = inputs["des"].shape[0]
    m.N = N
    m.ncores = ncores
    nsh = N // ncores
    assert nsh * ncores == N
    npad = _ceil_to(nsh, 128)
    m.nsh, m.npad = nsh, npad
    T = npad // 128
    m.ntiles = T
    m.node_chunk = node_chunk
    assert npad % node_chunk == 0

    # tile-aligned quarters of each core's shard
    q_sizes = []
    base = T // NQUART
    extra = T - base * NQUART
    for q in range(NQUART):
        q_sizes.append(base + (1 if q < extra else 0))
    m.q_sizes = q_sizes  # in tiles
    m.q_start = np.concatenate([[0], np.cumsum(q_sizes)])  # tile index
    m.q_rows = [s * 128 for s in q_sizes]  # rows per (core, rel) quarter
    m.q_tab_rows = [ncores * 2 * r for r in m.q_rows]  # rows per h_full_q
    assert all(r <= 32768 for r in m.q_tab_rows)

    src = np.asarray(inputs["edge_index"][0], dtype=np.int64)
    dst = np.asarray(inputs["edge_index"][1], dtype=np.int64)
    rel = np.asarray(inputs["edge_type"], dtype=np.int64)

    cnt = np.zeros((2, N), np.int64)
    for r in (0, 1):
        sel = rel == r
        cnt[r] = np.bincount(dst[sel], minlength=N)
    invc = 1.0 / np.maximum(cnt, 1).astype(np.float32)

    core_s, loc_s = src // nsh, src % nsh
    tile_s = loc_s // 128
    q_of_tile = np.zeros(T, np.int64)
    for q in range(NQUART):
        q_of_tile[m.q_start[q] : m.q_start[q + 1]] = q
    g_e = q_of_tile[tile_s]  # edge's gather group = src quarter
    qr = np.asarray(m.q_rows)[g_e]
    woff = loc_s - m.q_start[g_e] * 128
    idx16 = core_s * 2 * qr + rel * qr + woff  # row within h_full_q[g]
    assert idx16.max() < 32768
    coeff = invc[rel, dst]

    core_d = dst // nsh
    locd = dst % nsh
    tile_d = locd // 128
    dst7 = locd % 128

    counts = np.zeros((ncores, T, NQUART), np.int64)
    np.add.at(counts, (core_d, tile_d, g_e), 1)
    bud = counts.max(axis=0)
    bud = np.where(bud > 0, ((bud + 15) // 16) * 16, 0)  # 16-granular
    m.bud = bud
    m.gnb = -(-bud // 128)  # blocks per (t, g)

    # g-major streams: idx offsets and S-block offsets ordered (g, t)
    m.goff = np.zeros((T, NQUART), np.int64)
    m.nboff = np.zeros((T, NQUART), np.int64)
    ioff = 0
    boff = 0
    for g in range(NQUART):
        for t in range(T):
            m.goff[t, g] = ioff
            ioff += bud[t, g]
            m.nboff[t, g] = boff
            boff += m.gnb[t, g]
    m.totidx = int(ioff)
    m.totnb = int(boff)

    in_maps = []
    order = np.lexsort((idx16, tile_d, core_d, g_e))
    o_idx16, o_dst7, o_coeff = idx16[order], dst7[order], coeff[order]
    key = (g_e[order] * ncores + core_d[order]) * T + tile_d[order]
    uniq_start = np.searchsorted(key, np.arange(NQUART * ncores * T), "left")
    uniq_end = np.searchsorted(key, np.arange(NQUART * ncores * T), "right")

    des = np.asarray(inputs["des"], np.float32)
    tweet = np.asarray(inputs["tweet"], np.float32)
    nump = np.asarray(inputs["num_prop"], np.float32)
    catp = np.asarray(inputs["cat_prop"], np.float32)

    def shard_T(x, c, kblocks):
        xs = x[c * nsh : (c + 1) * nsh]
        out = np.zeros((kblocks * 128, npad), np.float16)
        out[: x.shape[1], :nsh] = xs.T
        return out.reshape(kblocks, 128, npad)

    def small_T(x, c, d):
        xs = x[c * nsh : (c + 1) * nsh]
        out = np.zeros((d, npad), np.float16)
        out[: x.shape[1], :nsh] = xs.T
        return out

    f16 = np.float16
    w = {}
    w["Wd6"] = np.ascontiguousarray(
        np.asarray(inputs["Wd"], f16).reshape(6, 128, 64)
    )
    w["Wt6"] = np.ascontiguousarray(
        np.asarray(inputs["Wt"], f16).reshape(6, 128, 64)
    )
    w["Wn"] = np.asarray(inputs["Wn"], f16)
    w["Wc"] = np.asarray(inputs["Wc"], f16)
    w["Wi2"] = np.asarray(inputs["Wi"], f16).reshape(2, 128, 256)
    w["W1"] = np.asarray(inputs["rel_w1"], f16).reshape(2, 2, 128, 256)
    w["root1"] = np.asarray(inputs["root_w1"], f16).reshape(2, 128, 256)
    w["W2"] = np.asarray(inputs["rel_w2"], f16).reshape(2, 2, 128, 256)
    w["root2"] = np.asarray(inputs["root_w2"], f16).reshape(2, 128, 256)
    w["Wo1"] = np.asarray(inputs["Wo1"], f16).reshape(2, 128, 256)
    w["Wo2"] = np.asarray(inputs["Wo2"], f16).reshape(2, 128, 2)
    w["bias_a"] = (
        np.concatenate([np.asarray(inputs["bd"]), np.asarray(inputs["bt"])])
        .astype(np.float32)
        .reshape(128, 1)
    )
    w["bias_b"] = (
        np.concatenate([np.asarray(inputs["bn"]), np.asarray(inputs["bc"])])
        .astype(np.float32)
        .reshape(128, 1)
    )
    w["bi_col"] = np.asarray(inputs["bi"], np.float32).reshape(2, 128).T.copy()
    w["bias1_rep"] = np.tile(
        np.asarray(inputs["bias1"], np.float32)[None, :], (128, 1)
    )
    w["bias2_rep"] = np.tile(
        np.asarray(inputs["bias2"], np.float32)[None, :], (128, 1)
    )
    w["bo1_col"] = np.asarray(inputs["bo1"], np.float32).reshape(2, 128).T.copy()
    w["bo2_rep"] = np.tile(
        np.asarray(inputs["bo2"], np.float32)[None, :], (128, 1)
    )
    w["ident"] = np.eye(128, dtype=f16)

    for c in range(ncores):
        idx_all = np.zeros((m.totidx,), np.int16)
        sblk = np.zeros((m.totnb, 128, 128), np.float16)
        for g in range(NQUART):
            for t in range(T):
                B = int(bud[t, g])
                if B == 0:
                    continue
                u = (g * ncores + c) * T + t
                s0, s1 = uniq_start[u], uniq_end[u]
                n = s1 - s0
                o = int(m.goff[t, g])
                idx_all[o : o + n] = o_idx16[s0:s1].astype(np.int16)
                e = np.arange(n)
                blk = int(m.nboff[t, g]) + e // 128
                sblk[blk, e % 128, o_dst7[s0:s1].astype(np.int64)] = o_coeff[
                    s0:s1
                ]
        idx_w = np.zeros((128, m.totidx // 16), np.int16)
        for g in range(NQUART):
            for t in range(T):
                B = int(bud[t, g])
                if B == 0:
                    continue
                o = int(m.goff[t, g])
                seg = idx_all[o : o + B].reshape(B // 16, 16).T
                idx_w[:, o // 16 : (o + B) // 16] = np.tile(seg, (8, 1))
        im = {
            "desT": shard_T(des, c, 6),
            "tweetT": shard_T(tweet, c, 6),
            "numT": small_T(nump, c, 5),
            "catT": small_T(catp, c, 3),
            "idx": idx_w,
            "sblk": np.ascontiguousarray(sblk.transpose(1, 0, 2)),
        }
        im.update(w)
        in_maps.append(im)
    return m, in_maps


# ---------------------------------------------------------------- builder
def build(m, debug=False, repeats=1, variant="full"):
    nq = 4 if "q4" in variant else 1
    nc = bacc.Bacc(
        "TRN2",
        target_bir_lowering=False,
        debug=debug,
        enable_asserts=True,
        num_devices=m.ncores,
        num_swdge_queues=nq,
    )
    npad, T = m.npad, m.ntiles
    NCH = m.node_chunk

    f32, f16, i16 = dt.float32, dt.float16, dt.int16
    ein, eout = "ExternalInput", "ExternalOutput"

    desT = nc.dram_tensor("desT", [6, 128, npad], f16, kind=ein)
    tweetT = nc.dram_tensor("tweetT", [6, 128, npad], f16, kind=ein)
    numT = nc.dram_tensor("numT", [5, npad], f16, kind=ein)
    catT = nc.dram_tensor("catT", [3, npad], f16, kind=ein)
    idx_ext = nc.dram_tensor("idx", [128, m.totidx // 16], i16, kind=ein)
    sblk_ext = nc.dram_tensor("sblk", [128, m.totnb, 128], f16, kind=ein)
    Wd6 = nc.dram_tensor("Wd6", [6, 128, 64], f16, kind=ein)
    Wt6 = nc.dram_tensor("Wt6", [6, 128, 64], f16, kind=ein)
    Wn = nc.dram_tensor("Wn", [5, 64], f16, kind=ein)
    Wc = nc.dram_tensor("Wc", [3, 64], f16, kind=ein)
    Wi2 = nc.dram_tensor("Wi2", [2, 128, 256], f16, kind=ein)
    W1 = nc.dram_tensor("W1", [2, 2, 128, 256], f16, kind=ein)
    root1 = nc.dram_tensor("root1", [2, 128, 256], f16, kind=ein)
    W2 = nc.dram_tensor("W2", [2, 2, 128, 256], f16, kind=ein)
    root2 = nc.dram_tensor("root2", [2, 128, 256], f16, kind=ein)
    Wo1 = nc.dram_tensor("Wo1", [2, 128, 256], f16, kind=ein)
    Wo2 = nc.dram_tensor("Wo2", [2, 128, 2], f16, kind=ein)
    bias_a = nc.dram_tensor("bias_a", [128, 1], f32, kind=ein)
    bias_b = nc.dram_tensor("bias_b", [128, 1], f32, kind=ein)
    bi_col = nc.dram_tensor("bi_col", [128, 2], f32, kind=ein)
    bias1_rep = nc.dram_tensor("bias1_rep", [128, 256], f32, kind=ein)
    bias2_rep = nc.dram_tensor("bias2_rep", [128, 256], f32, kind=ein)
    bo1_col = nc.dram_tensor("bo1_col", [128, 2], f32, kind=ein)
    bo2_rep = nc.dram_tensor("bo2_rep", [128, 2], f32, kind=ein)
    ident = nc.dram_tensor("ident", [128, 128], f16, kind=ein)
    out_ext = nc.dram_tensor("out", [npad, 2], f32, kind=eout)

    h1_loc = [
        nc.dram_tensor(f"h1_loc{q}", [2 * m.q_rows[q], 256], f16)
        for q in range(NQUART)
    ]
    h1_full = [
        nc.dram_tensor(
            f"h1_full{q}", [m.q_tab_rows[q], 256], f16, addr_space="Shared"
        )
        for q in range(NQUART)
    ]
    h2_loc = [
        nc.dram_tensor(f"h2_loc{q}", [2 * m.q_rows[q], 256], f16)
        for q in range(NQUART)
    ]
    h2_full = [
        nc.dram_tensor(
            f"h2_full{q}", [m.q_tab_rows[q], 256], f16, addr_space="Shared"
        )
        for q in range(NQUART)
    ]

    gnb_max = int(m.gnb.max())
    dma_sems = [nc.alloc_semaphore(f"gsem{q}") for q in range(nq)]

    def AG(loc, full):
        nc.gpsimd.collective_compute(
            "AllGather",
            mybir.AluOpType.bypass,
            ins=[loc[:]],
            outs=[full[:]],
            replica_groups=[list(range(m.ncores))],
        )

    def lrelu_from(pool, dst_ap, src_ap, bias_ap, shape):
        t0 = pool.tile(shape, f32, tag="lr0", name="lr0")
        nc.scalar.activation(
            t0[:], src_ap, mybir.ActivationFunctionType.Identity, bias=bias_ap
        )
        t1 = pool.tile(shape, f32, tag="lr1", name="lr1")
        nc.vector.tensor_scalar_mul(t1[:], t0[:], SLOPE)
        nc.vector.tensor_max(dst_ap, t0[:], t1[:])

    with tile.TileContext(nc) as tc:
        with (
            tc.tile_pool(name="wpool", bufs=1) as wp,
            tc.tile_pool(name="xres", bufs=1) as xres,
        ):
            wd_sb = wp.tile([128, 6, 64], f16)
            nc.sync.dma_start(wd_sb[:], _pmaj(Wd6))
            wt_sb = wp.tile([128, 6, 64], f16)
            nc.sync.dma_start(wt_sb[:], _pmaj(Wt6))
            wn_sb = wp.tile([5, 64], f16)
            nc.sync.dma_start(wn_sb[:], Wn[:])
            wc_sb = wp.tile([3, 64], f16)
            nc.sync.dma_start(wc_sb[:], Wc[:])
            wi_sb = wp.tile([128, 2, 256], f16)
            nc.sync.dma_start(wi_sb[:], _pmaj(Wi2))
            w1_sb = wp.tile([128, 4, 256], f16)
            nc.sync.dma_start(w1_sb[:], W1.ap().rearrange("r k p m -> p (r k) m"))
            r1_sb = wp.tile([128, 2, 256], f16)
            nc.sync.dma_start(r1_sb[:], _pmaj(root1))
            w2_sb = wp.tile([128, 4, 256], f16)
            nc.sync.dma_start(w2_sb[:], W2.ap().rearrange("r k p m -> p (r k) m"))
            r2_sb = wp.tile([128, 2, 256], f16)
            nc.sync.dma_start(r2_sb[:], _pmaj(root2))
            wo1_sb = wp.tile([128, 2, 256], f16)
            nc.sync.dma_start(wo1_sb[:], _pmaj(Wo1))
            wo2_sb = wp.tile([128, 2, 2], f16)
            nc.sync.dma_start(wo2_sb[:], _pmaj(Wo2))
            ba_sb = wp.tile([128, 1], f32)
            nc.sync.dma_start(ba_sb[:], bias_a[:])
            bb_sb = wp.tile([128, 1], f32)
            nc.sync.dma_start(bb_sb[:], bias_b[:])
            bi_sb = wp.tile([128, 2], f32)
            nc.sync.dma_start(bi_sb[:], bi_col[:])
            b1_sb = wp.tile([128, 256], f32)
            nc.sync.dma_start(b1_sb[:], bias1_rep[:])
            b2_sb = wp.tile([128, 256], f32)
            nc.sync.dma_start(b2_sb[:], bias2_rep[:])
            bo1_sb = wp.tile([128, 2], f32)
            nc.sync.dma_start(bo1_sb[:], bo1_col[:])
            bo2_sb = wp.tile([128, 2], f32)
            nc.sync.dma_start(bo2_sb[:], bo2_rep[:])
            id_sb = wp.tile([128, 128], f16)
            nc.sync.dma_start(id_sb[:], ident[:])
            idx_sb = wp.tile([128, m.totidx // 16], i16)
            nc.sync.dma_start(idx_sb[:], idx_ext[:])

            x1a = xres.tile([128, npad], f16, tag="x1a")
            x1b = xres.tile([128, npad], f16, tag="x1b")
            x2a = xres.tile([128, npad], f16, tag="x2a")
            x2b = xres.tile([128, npad], f16, tag="x2b")
            acc = xres.tile([128, T, 256], f32, tag="acc")
            out_stage = xres.tile([128, T, 2], f32, tag="outst")

            def whole_body():
                # -------- phase 0: feature pipeline -> x1T --------
                with (
                    tc.tile_pool(name="p0", bufs=2) as p0,
                    tc.tile_pool(name="p0ps", bufs=2, space="PSUM") as p0ps,
                ):
                    for c0 in range(0, npad, NCH):
                        dsb = p0.tile([128, 6, NCH], f16, tag="des", name="dsb")
                        nc.sync.dma_start(
                            dsb[:],
                            desT.ap()[:, :, c0 : c0 + NCH].rearrange(
                                "k p n -> p k n"
                            ),
                        )
                        tsb = p0.tile([128, 6, NCH], f16, tag="tw", name="tsb")
                        nc.sync.dma_start(
                            tsb[:],
                            tweetT.ap()[:, :, c0 : c0 + NCH].rearrange(
                                "k p n -> p k n"
                            ),
                        )
                        nsb = p0.tile([5, NCH], f16, tag="np", name="nsb")
                        nc.sync.dma_start(nsb[:], numT.ap()[:, c0 : c0 + NCH])
                        csb = p0.tile([3, NCH], f16, tag="cp", name="csb")
                        nc.sync.dma_start(csb[:], catT.ap()[:, c0 : c0 + NCH])

                        ps_a = p0ps.tile([128, NCH], f32, tag="psa", name="ps_a")
                        for k in range(6):
                            nc.tensor.matmul(
                                ps_a[0:64, :],
                                wd_sb[:, k, :],
                                dsb[:, k, :],
                                start=(k == 0),
                                stop=(k == 5),
                            )
                        for k in range(6):
                            nc.tensor.matmul(
                                ps_a[64:128, :],
                                wt_sb[:, k, :],
                                tsb[:, k, :],
                                start=(k == 0),
                                stop=(k == 5),
                                tile_position=(0, 64),
                            )
                        ps_b = p0ps.tile([128, NCH], f32, tag="psb", name="ps_b")
                        nc.tensor.matmul(
                            ps_b[0:64, :], wn_sb[:], nsb[:], start=True, stop=True
                        )
                        nc.tensor.matmul(
                            ps_b[64:128, :],
                            wc_sb[:],
                            csb[:],
                            start=True,
                            stop=True,
                            tile_position=(0, 64),
                        )
                        x0a = p0.tile([128, NCH], f16, tag="x0a", name="x0a")
                        lrelu_from(p0, x0a[:], ps_a[:], ba_sb[:], [128, NCH])
                        x0b = p0.tile([128, NCH], f16, tag="x0b", name="x0b")
                        lrelu_from(p0, x0b[:], ps_b[:], bb_sb[:], [128, NCH])

                        for h, xdst in ((0, x1a), (1, x1b)):
                            ps_x = p0ps.tile(
                                [128, NCH], f32, tag="psx", name="ps_x"
                            )
                            nc.tensor.matmul(
                                ps_x[:],
                                wi_sb[:, 0, h * 128 : (h + 1) * 128],
                                x0a[:],
                                start=True,
                                stop=False,
                            )
                            nc.tensor.matmul(
                                ps_x[:],
                                wi_sb[:, 1, h * 128 : (h + 1) * 128],
                                x0b[:],
                                start=False,
                                stop=True,
                            )
                            lrelu_from(
                                p0,
                                xdst[:, c0 : c0 + NCH],
                                ps_x[:],
                                bi_sb[:, h : h + 1],
                                [128, NCH],
                            )

                # h production into quarter-major local tables
                def produce_h(xa, xb, w_sb, h_loc_q, pool, pps):
                    for q in range(NQUART):
                        qt0 = int(m.q_start[q])
                        qt1 = int(m.q_start[q + 1])
                        for r in range(2):
                            for t0 in range(qt0, qt1, 4):
                                nt = min(4, qt1 - t0)
                                hsb = pool.tile(
                                    [128, 4, 256], f16, tag="hsb", name="hsb"
                                )
                                for j in range(nt):
                                    t = t0 + j
                                    ts = slice(t * 128, (t + 1) * 128)
                                    ph = pps.tile(
                                        [128, 256], f32, tag="ph", name="ph"
                                    )
                                    nc.tensor.matmul(
                                        ph[:],
                                        xa[:, ts],
                                        w_sb[:, 2 * r, :],
                                        start=True,
                                        stop=False,
                                    )
                                    nc.tensor.matmul(
                                        ph[:],
                                        xb[:, ts],
                                        w_sb[:, 2 * r + 1, :],
                                        start=False,
                                        stop=True,
                                    )
                                    nc.vector.tensor_copy(hsb[:, j, :], ph[:])
                                row0 = r * m.q_rows[q] + (t0 - qt0) * 128
                                nc.sync.dma_start(
                                    h_loc_q[q]
                                    .ap()[row0 : row0 + nt * 128, :]
                                    .rearrange("(b p) f -> p b f", p=128),
                                    hsb[:, 0:nt, :],
                                )

                with (
                    tc.tile_pool(name="hp", bufs=3) as hp,
                    tc.tile_pool(name="hpps", bufs=2, space="PSUM") as hpps,
                ):
                    produce_h(x1a, x1b, w1_sb, h1_loc, hp, hpps)
                for q in range(NQUART):
                    AG(h1_loc[q], h1_full[q])

                # -------- RGCN layer: quarter-outer aggregation --------
                def rgcn_layer(xa, xb, r_sb, b_sb, h_full_q, out_cb, lp, lps):
                    # root + bias -> acc (runs during the AllGathers)
                    for t in range(T):
                        ts = slice(t * 128, (t + 1) * 128)
                        rpo = lps.tile([128, 256], f32, tag="paux", name="rpo")
                        nc.tensor.matmul(
                            rpo[:], xa[:, ts], r_sb[:, 0, :],
                            start=True, stop=False,
                        )
                        nc.tensor.matmul(
                            rpo[:], xb[:, ts], r_sb[:, 1, :],
                            start=False, stop=True,
                        )
                        nc.vector.tensor_add(acc[:, t, :], rpo[:], b_sb[:])
                    # aggregation, quarter-outer so AG(q+1) overlaps work(q);
                    # outputs are emitted inline during the last quarter.
                    for g in range(NQUART):
                        rows_q = m.q_tab_rows[g]
                        last_q = g == NQUART - 1
                        for t in range(T):
                            B = int(m.bud[t, g])
                            if B > 0:
                                nbg = int(m.gnb[t, g])
                                nboff = int(m.nboff[t, g])
                                o = int(m.goff[t, g])
                                qn = t % nq
                                ssb = lp.tile(
                                    [128, gnb_max, 128], f16,
                                    tag="ssb", name="ssb", bufs=3,
                                )
                                nc.sync.dma_start(
                                    ssb[:, 0:nbg, :],
                                    sblk_ext.ap()[:, nboff : nboff + nbg, :],
                                )
                                msg = lp.tile(
                                    [128, gnb_max, 256], f16,
                                    tag=f"msg{qn}", name=f"msg{qn}", bufs=3,
                                )
                                nc.gpsimd.dma_gather(
                                    msg[:, 0:nbg, :],
                                    h_full_q[g].ap()[0:rows_q, :],
                                    idx_sb[:, o // 16 : (o + B) // 16],
                                    num_idxs=B,
                                    num_idxs_reg=B,
                                    elem_size=256,
                                    single_packet="sp0" not in variant,
                                    queue_num=qn,
                                )
                                po = lps.tile(
                                    [128, 256], f32, tag="po", name="po",
                                    bufs=4,
                                )
                                for b in range(nbg):
                                    K = min(128, B - b * 128)
                                    nc.tensor.matmul(
                                        po[:],
                                        ssb[0:K, b, :],
                                        msg[0:K, b, :],
                                        start=(b == 0),
                                        stop=(b == nbg - 1),
                                    )
                                nc.vector.tensor_add(
                                    acc[:, t, :], acc[:, t, :], po[:]
                                )
                            if last_q:
                                ts = slice(t * 128, (t + 1) * 128)
                                osb = lp.tile(
                                    [128, 256], f16, tag="osb", name="osb"
                                )
                                nc.vector.tensor_copy(osb[:], acc[:, t, :])
                                out_cb(t, ts, osb, lp, lps)

                h2_stage = [None, None]

                def l1_out(t, ts, osb, lp, lps):
                    for h, xdst in ((0, x2a), (1, x2b)):
                        pt = lps.tile([128, 128], f16, tag="pt", name="pt")
                        nc.tensor.transpose(
                            pt[:], osb[:, h * 128 : (h + 1) * 128], id_sb[:]
                        )
                        nc.vector.tensor_copy(xdst[:, ts], pt[:])
                    j = t % 4
                    if j == 0:
                        h2_stage[0] = lp.tile(
                            [128, 4, 256], f16, tag="h2s0", name="h2s0"
                        )
                        h2_stage[1] = lp.tile(
                            [128, 4, 256], f16, tag="h2s1", name="h2s1"
                        )
                    for r in range(2):
                        ph = lps.tile([128, 256], f32, tag="paux", name="ph2")
                        nc.tensor.matmul(
                            ph[:], x2a[:, ts], w2_sb[:, 2 * r, :],
                            start=True, stop=False,
                        )
                        nc.tensor.matmul(
                            ph[:], x2b[:, ts], w2_sb[:, 2 * r + 1, :],
                            start=False, stop=True,
                        )
                        nc.vector.tensor_copy(h2_stage[r][:, j, :], ph[:])
                    # flush h2 stage at quarter boundaries (or every 4 tiles)
                    qq = int(np.searchsorted(m.q_start, t, "right")) - 1
                    qt0 = int(m.q_start[qq])
                    qt1 = int(m.q_start[qq + 1])
                    if j == 3 or t == qt1 - 1:
                        t0 = t - j
                        nt = j + 1
                        # may straddle a quarter boundary: split per quarter
                        tcur = t0
                        while tcur <= t:
                            q2 = int(np.searchsorted(m.q_start, tcur, "right")) - 1
                            q2end = int(m.q_start[q2 + 1])
                            tstop = min(t, q2end - 1)
                            cnt_t = tstop - tcur + 1
                            for r in range(2):
                                row0 = (
                                    r * m.q_rows[q2]
                                    + (tcur - int(m.q_start[q2])) * 128
                                )
                                nc.sync.dma_start(
                                    h2_loc[q2]
                                    .ap()[row0 : row0 + cnt_t * 128, :]
                                    .rearrange("(b p) f -> p b f", p=128),
                                    h2_stage[r][:, tcur - t0 : tcur - t0 + cnt_t, :],
                                )
                            tcur = tstop + 1
                    if t == qt1 - 1:
                        AG(h2_loc[qq], h2_full[qq])

                def l2_out(t, ts, osb, lp, lps):
                    o2t = lp.tile([128, 2, 128], f16, tag="o2t", name="o2t")
                    for h in range(2):
                        pt = lps.tile([128, 128], f16, tag="pt", name="pt")
                        nc.tensor.transpose(
                            pt[:], osb[:, h * 128 : (h + 1) * 128], id_sb[:]
                        )
                        nc.vector.tensor_copy(o2t[:, h, :], pt[:])
                    ht = lp.tile([128, 2, 128], f16, tag="ht", name="ht")
                    for h in range(2):
                        phd = lps.tile([128, 256], f32, tag="paux", name="phd")
                        nc.tensor.matmul(
                            phd[:, 0:128],
                            wo1_sb[:, 0, h * 128 : (h + 1) * 128],
                            o2t[:, 0, :],
                            start=True,
                            stop=False,
                        )
                        nc.tensor.matmul(
                            phd[:, 0:128],
                            wo1_sb[:, 1, h * 128 : (h + 1) * 128],
                            o2t[:, 1, :],
                            start=False,
                            stop=True,
                        )
                        lrelu_from(
                            lp, ht[:, h, :], phd[:, 0:128], bo1_sb[:, h : h + 1],
                            [128, 128],
                        )
                    pf = lps.tile([128, 256], f32, tag="paux", name="pf")
                    nc.tensor.matmul(
                        pf[:, 0:2], ht[:, 0, :], wo2_sb[:, 0, :],
                        start=True, stop=False,
                    )
                    nc.tensor.matmul(
                        pf[:, 0:2], ht[:, 1, :], wo2_sb[:, 1, :],
                        start=False, stop=True,
                    )
                    nc.vector.tensor_add(
                        out_stage[:, t, :], pf[:, 0:2], bo2_sb[:]
                    )
                    if t == T - 1:
                        nc.sync.dma_start(
                            out_ext.ap().rearrange("(b p) f -> p b f", p=128),
                            out_stage[:],
                        )

                with (
                    tc.tile_pool(name="lyr", bufs=2) as lp,
                    tc.tile_pool(name="lyrps", bufs=2, space="PSUM") as lps,
                ):
                    rgcn_layer(
                        x1a, x1b, r1_sb, b1_sb, h1_full, l1_out, lp, lps
                    )
                    rgcn_layer(
                        x2a, x2b, r2_sb, b2_sb, h2_full, l2_out, lp, lps
                    )

            for _rep in range(repeats):
                whole_body()

    nc.compile()
    return nc


def _pmaj(t):
    """DRAM tensor [a, 128, b] viewed partition-major [128, a, b]."""
    return t.ap().rearrange("a p b -> p a b")


# ---------------------------------------------------------------- entry
def kernel(**inputs):
    meta, in_maps = prepare(inputs)
    nc = build(meta, variant=DEFAULT_VARIANT)
    from concourse.bass_utils import run_bass_kernel_spmd

    res = run_bass_kernel_spmd(
        nc, in_maps, core_ids=list(range(meta.ncores))
    ).results
    out = np.concatenate(
        [res[c]["out"][: meta.nsh] for c in range(meta.ncores)], axis=0
    )
    return out.astype(np.float32)


# revision 19
# speedup vs baseline: 1.0615x; 1.0615x over previous
"""BotRGCN on 8 Trainium2 NeuronCores (Bass/Tile, SPMD).

Strategy (per sharding hint): nodes row-sharded across 8 cores, edges
partitioned by destination node. Per RGCN layer, each core computes the
per-relation transformed features h_r = x @ W_r for its node shard
(node-major, f16). The h table is exchanged with FOUR pipelined
AllGathers (one per tile-aligned quarter of the node shard) so that
message aggregation for quarter g overlaps the collective for quarter
g+1. Each core aggregates messages for its destination shard with
prepared indexed DMA row-gathers (descriptor emission decoupled from
the h dependency via prepare_only/trigger_dma) followed by one-hot
S-matrix matmuls on the tensor engine that perform the segment-mean
(1/cnt folded into S). S blocks are precomputed on the host and
streamed from HBM (f16). Aggregation runs quarter-outer into an SBUF
f32 accumulator; root terms are computed directly into the accumulator
during the AllGathers. All dense matmuls run in f16 (f32 PSUM).

Self-contained: hardcodes problem shapes; the host side shards inputs,
builds gather indices + S blocks, compiles one SPMD Bass program and
runs it on cores 0-7.
"""
import sys

import numpy as np

for _p in ("/opt/trn_rl_repo",):
    if _p not in sys.path:
        sys.path.insert(0, _p)

import concourse.bacc as bacc
import concourse.mybir as mybir
from concourse import tile

dt = mybir.dt

NCORES = 8
SLOPE = 0.01
DEFAULT_VARIANT = "sp0q4"
NQUART = 4


def _ceil_to(x, m):
    return ((x + m - 1) // m) * m


class Meta:
    pass


def prepare(inputs, ncores=NCORES, node_chunk=448):
    """Shard inputs, build gather indices + host-precomputed S blocks."""
    m = Meta()
    N = inputs["des"].shape[0]
    m.N = N
    m.ncores = ncores
    nsh = N // ncores
    assert nsh * ncores == N
    npad = _ceil_to(nsh, 128)
    m.nsh, m.npad = nsh, npad
    T = npad // 128
    m.ntiles = T
    m.node_chunk = node_chunk
    assert npad % node_chunk == 0

    # tile-aligned quarters of each core's shard
    q_sizes = []
    base = T // NQUART
    extra = T - base * NQUART
    for q in range(NQUART):
        q_sizes.append(base + (1 if q < extra else 0))
    m.q_sizes = q_sizes  # in tiles
    m.q_start = np.concatenate([[0], np.cumsum(q_sizes)])  # tile index
    m.q_rows = [s * 128 for s in q_sizes]  # rows per (core, rel) quarter
    m.q_tab_rows = [ncores * 2 * r for r in m.q_rows]  # rows per h_full_q
    assert all(r <= 32768 for r in m.q_tab_rows)

    src = np.asarray(inputs["edge_index"][0], dtype=np.int64)
    dst = np.asarray(inputs["edge_index"][1], dtype=np.int64)
    rel = np.asarray(inputs["edge_type"], dtype=np.int64)

    cnt = np.zeros((2, N), np.int64)
    for r in (0, 1):
        sel = rel == r
        cnt[r] = np.bincount(dst[sel], minlength=N)
    invc = 1.0 / np.maximum(cnt, 1).astype(np.float32)

    core_s, loc_s = src // nsh, src % nsh
    tile_s = loc_s // 128
    q_of_tile = np.zeros(T, np.int64)
    for q in range(NQUART):
        q_of_tile[m.q_start[q] : m.q_start[q + 1]] = q
    g_e = q_of_tile[tile_s]  # edge's gather group = src quarter
    qr = np.asarray(m.q_rows)[g_e]
    woff = loc_s - m.q_start[g_e] * 128
    idx16 = core_s * 2 * qr + rel * qr + woff  # row within h_full_q[g]
    assert idx16.max() < 32768
    coeff = invc[rel, dst]

    core_d = dst // nsh
    locd = dst % nsh
    tile_d = locd // 128
    dst7 = locd % 128

    counts = np.zeros((ncores, T, NQUART), np.int64)
    np.add.at(counts, (core_d, tile_d, g_e), 1)
    bud = counts.max(axis=0)
    bud = np.where(bud > 0, ((bud + 15) // 16) * 16, 0)  # 16-granular
    m.bud = bud
    m.gnb = -(-bud // 128)  # blocks per (t, g)

    # g-major streams: idx offsets and S-block offsets ordered (g, t)
    m.goff = np.zeros((T, NQUART), np.int64)
    m.nboff = np.zeros((T, NQUART), np.int64)
    ioff = 0
    boff = 0
    for g in range(NQUART):
        for t in range(T):
            m.goff[t, g] = ioff
            ioff += bud[t, g]
            m.nboff[t, g] = boff
            boff += m.gnb[t, g]
    m.totidx = int(ioff)
    m.totnb = int(boff)

    in_maps = []
    order = np.lexsort((idx16, tile_d, core_d, g_e))
    o_idx16, o_dst7, o_coeff = idx16[order], dst7[order], coeff[order]
    key = (g_e[order] * ncores + core_d[order]) * T + tile_d[order]
    uniq_start = np.searchsorted(key, np.arange(NQUART * ncores * T), "left")
    uniq_end = np.searchsorted(key, np.arange(NQUART * ncores * T), "right")

    des = np.asarray(inputs["des"], np.float32)
    tweet = np.asarray(inputs["tweet"], np.float32)
    nump = np.asarray(inputs["num_prop"], np.float32)
    catp = np.asarray(inputs["cat_prop"], np.float32)

    def shard_T(x, c, kblocks):
        xs = x[c * nsh : (c + 1) * nsh]
        out = np.zeros((kblocks * 128, npad), np.float16)
        out[: x.shape[1], :nsh] = xs.T
        return out.reshape(kblocks, 128, npad)

    def small_T(x, c, d):
        xs = x[c * nsh : (c + 1) * nsh]
        out = np.zeros((d, npad), np.float16)
        out[: x.shape[1], :nsh] = xs.T
        return out

    f16 = np.float16
    w = {}
    w["Wd6"] = np.ascontiguousarray(
        np.asarray(inputs["Wd"], f16).reshape(6, 128, 64)
    )
    w["Wt6"] = np.ascontiguousarray(
        np.asarray(inputs["Wt"], f16).reshape(6, 128, 64)
    )
    w["Wn"] = np.asarray(inputs["Wn"], f16)
    w["Wc"] = np.asarray(inputs["Wc"], f16)
    w["Wi2"] = np.asarray(inputs["Wi"], f16).reshape(2, 128, 256)
    w["W1"] = np.asarray(inputs["rel_w1"], f16).reshape(2, 2, 128, 256)
    w["root1"] = np.asarray(inputs["root_w1"], f16).reshape(2, 128, 256)
    w["W2"] = np.asarray(inputs["rel_w2"], f16).reshape(2, 2, 128, 256)
    w["root2"] = np.asarray(inputs["root_w2"], f16).reshape(2, 128, 256)
    w["Wo1"] = np.asarray(inputs["Wo1"], f16).reshape(2, 128, 256)
    w["Wo2"] = np.asarray(inputs["Wo2"], f16).reshape(2, 128, 2)
    w["bias_a"] = (
        np.concatenate([np.asarray(inputs["bd"]), np.asarray(inputs["bt"])])
        .astype(np.float32)
        .reshape(128, 1)
    )
    w["bias_b"] = (
        np.concatenate([np.asarray(inputs["bn"]), np.asarray(inputs["bc"])])
        .astype(np.float32)
        .reshape(128, 1)
    )
    w["bi_col"] = np.asarray(inputs["bi"], np.float32).reshape(2, 128).T.copy()
    w["bias1_rep"] = np.tile(
        np.asarray(inputs["bias1"], np.float32)[None, :], (128, 1)
    )
    w["bias2_rep"] = np.tile(
        np.asarray(inputs["bias2"], np.float32)[None, :], (128, 1)
    )
    w["bo1_col"] = np.asarray(inputs["bo1"], np.float32).reshape(2, 128).T.copy()
    w["bo2_rep"] = np.tile(
        np.asarray(inputs["bo2"], np.float32)[None, :], (128, 1)
    )
    w["ident"] = np.eye(128, dtype=f16)

    for c in range(ncores):
        idx_all = np.zeros((m.totidx,), np.int16)
        sblk = np.zeros((m.totnb, 128, 128), np.float16)
        for g in range(NQUART):
            for t in range(T):
                B = int(bud[t, g])
                if B == 0:
                    continue
                u = (g * ncores + c) * T + t
                s0, s1 = uniq_start[u], uniq_end[u]
                n = s1 - s0
                o = int(m.goff[t, g])
                idx_all[o : o + n] = o_idx16[s0:s1].astype(np.int16)
                e = np.arange(n)
                blk = int(m.nboff[t, g]) + e // 128
                sblk[blk, e % 128, o_dst7[s0:s1].astype(np.int64)] = o_coeff[
                    s0:s1
                ]
        idx_w = np.zeros((128, m.totidx // 16), np.int16)
        for g in range(NQUART):
            for t in range(T):
                B = int(bud[t, g])
                if B == 0:
                    continue
                o = int(m.goff[t, g])
                seg = idx_all[o : o + B].reshape(B // 16, 16).T
                idx_w[:, o // 16 : (o + B) // 16] = np.tile(seg, (8, 1))
        im = {
            "desT": shard_T(des, c, 6),
            "tweetT": shard_T(tweet, c, 6),
            "numT": small_T(nump, c, 5),
            "catT": small_T(catp, c, 3),
            "idx": idx_w,
            "sblk": np.ascontiguousarray(sblk.transpose(1, 0, 2)),
        }
        im.update(w)
        in_maps.append(im)
    return m, in_maps


# ---------------------------------------------------------------- builder
def build(m, debug=False, repeats=1, variant="full"):
    nq = 4 if "q4" in variant else 1
    nc = bacc.Bacc(
        "TRN2",
        target_bir_lowering=False,
        debug=debug,
        enable_asserts=True,
        num_devices=m.ncores,
        num_swdge_queues=nq,
    )
    npad, T = m.npad, m.ntiles
    NCH = m.node_chunk

    f32, f16, i16 = dt.float32, dt.float16, dt.int16
    ein, eout = "ExternalInput", "ExternalOutput"

    desT = nc.dram_tensor("desT", [6, 128, npad], f16, kind=ein)
    tweetT = nc.dram_tensor("tweetT", [6, 128, npad], f16, kind=ein)
    numT = nc.dram_tensor("numT", [5, npad], f16, kind=ein)
    catT = nc.dram_tensor("catT", [3, npad], f16, kind=ein)
    idx_ext = nc.dram_tensor("idx", [128, m.totidx // 16], i16, kind=ein)
    sblk_ext = nc.dram_tensor("sblk", [128, m.totnb, 128], f16, kind=ein)
    Wd6 = nc.dram_tensor("Wd6", [6, 128, 64], f16, kind=ein)
    Wt6 = nc.dram_tensor("Wt6", [6, 128, 64], f16, kind=ein)
    Wn = nc.dram_tensor("Wn", [5, 64], f16, kind=ein)
    Wc = nc.dram_tensor("Wc", [3, 64], f16, kind=ein)
    Wi2 = nc.dram_tensor("Wi2", [2, 128, 256], f16, kind=ein)
    W1 = nc.dram_tensor("W1", [2, 2, 128, 256], f16, kind=ein)
    root1 = nc.dram_tensor("root1", [2, 128, 256], f16, kind=ein)
    W2 = nc.dram_tensor("W2", [2, 2, 128, 256], f16, kind=ein)
    root2 = nc.dram_tensor("root2", [2, 128, 256], f16, kind=ein)
    Wo1 = nc.dram_tensor("Wo1", [2, 128, 256], f16, kind=ein)
    Wo2 = nc.dram_tensor("Wo2", [2, 128, 2], f16, kind=ein)
    bias_a = nc.dram_tensor("bias_a", [128, 1], f32, kind=ein)
    bias_b = nc.dram_tensor("bias_b", [128, 1], f32, kind=ein)
    bi_col = nc.dram_tensor("bi_col", [128, 2], f32, kind=ein)
    bias1_rep = nc.dram_tensor("bias1_rep", [128, 256], f32, kind=ein)
    bias2_rep = nc.dram_tensor("bias2_rep", [128, 256], f32, kind=ein)
    bo1_col = nc.dram_tensor("bo1_col", [128, 2], f32, kind=ein)
    bo2_rep = nc.dram_tensor("bo2_rep", [128, 2], f32, kind=ein)
    ident = nc.dram_tensor("ident", [128, 128], f16, kind=ein)
    out_ext = nc.dram_tensor("out", [npad, 2], f32, kind=eout)

    h1_loc = [
        nc.dram_tensor(f"h1_loc{q}", [2 * m.q_rows[q], 256], f16)
        for q in range(NQUART)
    ]
    h1_full = [
        nc.dram_tensor(
            f"h1_full{q}", [m.q_tab_rows[q], 256], f16, addr_space="Shared"
        )
        for q in range(NQUART)
    ]
    h2_loc = [
        nc.dram_tensor(f"h2_loc{q}", [2 * m.q_rows[q], 256], f16)
        for q in range(NQUART)
    ]
    h2_full = [
        nc.dram_tensor(
            f"h2_full{q}", [m.q_tab_rows[q], 256], f16, addr_space="Shared"
        )
        for q in range(NQUART)
    ]

    gnb_max = int(m.gnb.max())
    dma_sems = [nc.alloc_semaphore(f"gsem{q}") for q in range(nq)]

    def AG(loc, full):
        nc.gpsimd.collective_compute(
            "AllGather",
            mybir.AluOpType.bypass,
            ins=[loc[:]],
            outs=[full[:]],
            replica_groups=[list(range(m.ncores))],
        )

    def lrelu_from(pool, dst_ap, src_ap, bias_ap, shape):
        w = shape[1]
        t0 = pool.tile([128, 448], f32, tag="lr0", name="lr0")
        nc.scalar.activation(
            t0[:, 0:w], src_ap, mybir.ActivationFunctionType.Identity,
            bias=bias_ap,
        )
        t1 = pool.tile([128, 448], f32, tag="lr1", name="lr1")
        nc.vector.tensor_scalar_mul(t1[:, 0:w], t0[:, 0:w], SLOPE)
        nc.vector.tensor_max(dst_ap, t0[:, 0:w], t1[:, 0:w])

    with tile.TileContext(nc) as tc:
        with (
            tc.tile_pool(name="wpool", bufs=1) as wp,
            tc.tile_pool(name="xres", bufs=1) as xres,
        ):
            wd_sb = wp.tile([128, 6, 64], f16)
            nc.sync.dma_start(wd_sb[:], _pmaj(Wd6))
            wt_sb = wp.tile([128, 6, 64], f16)
            nc.sync.dma_start(wt_sb[:], _pmaj(Wt6))
            wn_sb = wp.tile([5, 64], f16)
            nc.sync.dma_start(wn_sb[:], Wn[:])
            wc_sb = wp.tile([3, 64], f16)
            nc.sync.dma_start(wc_sb[:], Wc[:])
            wi_sb = wp.tile([128, 2, 256], f16)
            nc.sync.dma_start(wi_sb[:], _pmaj(Wi2))
            w1_sb = wp.tile([128, 4, 256], f16)
            nc.sync.dma_start(w1_sb[:], W1.ap().rearrange("r k p m -> p (r k) m"))
            r1_sb = wp.tile([128, 2, 256], f16)
            nc.sync.dma_start(r1_sb[:], _pmaj(root1))
            w2_sb = wp.tile([128, 4, 256], f16)
            nc.sync.dma_start(w2_sb[:], W2.ap().rearrange("r k p m -> p (r k) m"))
            r2_sb = wp.tile([128, 2, 256], f16)
            nc.sync.dma_start(r2_sb[:], _pmaj(root2))
            wo1_sb = wp.tile([128, 2, 256], f16)
            nc.sync.dma_start(wo1_sb[:], _pmaj(Wo1))
            wo2_sb = wp.tile([128, 2, 2], f16)
            nc.sync.dma_start(wo2_sb[:], _pmaj(Wo2))
            ba_sb = wp.tile([128, 1], f32)
            nc.sync.dma_start(ba_sb[:], bias_a[:])
            bb_sb = wp.tile([128, 1], f32)
            nc.sync.dma_start(bb_sb[:], bias_b[:])
            bi_sb = wp.tile([128, 2], f32)
            nc.sync.dma_start(bi_sb[:], bi_col[:])
            b1_sb = wp.tile([128, 256], f32)
            nc.sync.dma_start(b1_sb[:], bias1_rep[:])
            b2_sb = wp.tile([128, 256], f32)
            nc.sync.dma_start(b2_sb[:], bias2_rep[:])
            bo1_sb = wp.tile([128, 2], f32)
            nc.sync.dma_start(bo1_sb[:], bo1_col[:])
            bo2_sb = wp.tile([128, 2], f32)
            nc.sync.dma_start(bo2_sb[:], bo2_rep[:])
            id_sb = wp.tile([128, 128], f16)
            nc.sync.dma_start(id_sb[:], ident[:])
            idx_sb = wp.tile([128, m.totidx // 16], i16)
            nc.sync.dma_start(idx_sb[:], idx_ext[:])

            x1a = [
                xres.tile([128, m.q_rows[q]], f16, tag=f"x1a{q}", name=f"x1a{q}")
                for q in range(NQUART)
            ]
            x1b = [
                xres.tile([128, m.q_rows[q]], f16, tag=f"x1b{q}", name=f"x1b{q}")
                for q in range(NQUART)
            ]
            x2a = xres.tile([128, npad], f16, tag="x2a")
            x2b = xres.tile([128, npad], f16, tag="x2b")
            acc = xres.tile([128, T, 256], f32, tag="acc")
            out_stage = xres.tile([128, T, 2], f32, tag="outst")

            def whole_body():
                # -------- phase 0: feature pipeline -> x1 (per quarter),
                # with h1 production + its AllGather launched per quarter ----
                def produce_h_quarter(q, xa_q, xb_q, w_sb, h_loc_q, pool, pps):
                    nqt = m.q_sizes[q]
                    for r in range(2):
                        for lt0 in range(0, nqt, 4):
                            nt = min(4, nqt - lt0)
                            hsb = pool.tile(
                                [128, 4, 256], f16, tag="hsb", name="hsb"
                            )
                            for j in range(nt):
                                lts = slice((lt0 + j) * 128, (lt0 + j + 1) * 128)
                                ph = pps.tile(
                                    [128, 256], f32, tag="ph", name="ph"
                                )
                                nc.tensor.matmul(
                                    ph[:], xa_q[:, lts], w_sb[:, 2 * r, :],
                                    start=True, stop=False,
                                )
                                nc.tensor.matmul(
                                    ph[:], xb_q[:, lts], w_sb[:, 2 * r + 1, :],
                                    start=False, stop=True,
                                )
                                nc.vector.tensor_copy(hsb[:, j, :], ph[:])
                            row0 = r * m.q_rows[q] + lt0 * 128
                            nc.sync.dma_start(
                                h_loc_q[q]
                                .ap()[row0 : row0 + nt * 128, :]
                                .rearrange("(b p) f -> p b f", p=128),
                                hsb[:, 0:nt, :],
                            )

                with (
                    tc.tile_pool(name="p0", bufs=2) as p0,
                    tc.tile_pool(name="p0ps", bufs=2, space="PSUM") as p0ps,
                ):
                    for q in range(NQUART):
                        qr = m.q_rows[q]
                        gbase = int(m.q_start[q]) * 128
                        c0 = 0
                        while c0 < qr:
                            nch = min(NCH, qr - c0)
                            gc0 = gbase + c0
                            dsb = p0.tile(
                                [128, 6, NCH], f16, tag="des", name="dsb"
                            )
                            nc.sync.dma_start(
                                dsb[:, :, 0:nch],
                                desT.ap()[:, :, gc0 : gc0 + nch].rearrange(
                                    "k p n -> p k n"
                                ),
                            )
                            tsb = p0.tile(
                                [128, 6, NCH], f16, tag="tw", name="tsb"
                            )
                            nc.sync.dma_start(
                                tsb[:, :, 0:nch],
                                tweetT.ap()[:, :, gc0 : gc0 + nch].rearrange(
                                    "k p n -> p k n"
                                ),
                            )
                            nsb = p0.tile([5, NCH], f16, tag="np", name="nsb")
                            nc.sync.dma_start(
                                nsb[:, 0:nch], numT.ap()[:, gc0 : gc0 + nch]
                            )
                            csb = p0.tile([3, NCH], f16, tag="cp", name="csb")
                            nc.sync.dma_start(
                                csb[:, 0:nch], catT.ap()[:, gc0 : gc0 + nch]
                            )

                            ps_a = p0ps.tile(
                                [128, NCH], f32, tag="psa", name="ps_a"
                            )
                            for k in range(6):
                                nc.tensor.matmul(
                                    ps_a[0:64, 0:nch],
                                    wd_sb[:, k, :],
                                    dsb[:, k, 0:nch],
                                    start=(k == 0),
                                    stop=(k == 5),
                                )
                            for k in range(6):
                                nc.tensor.matmul(
                                    ps_a[64:128, 0:nch],
                                    wt_sb[:, k, :],
                                    tsb[:, k, 0:nch],
                                    start=(k == 0),
                                    stop=(k == 5),
                                    tile_position=(0, 64),
                                )
                            ps_b = p0ps.tile(
                                [128, NCH], f32, tag="psb", name="ps_b"
                            )
                            nc.tensor.matmul(
                                ps_b[0:64, 0:nch], wn_sb[:], nsb[:, 0:nch],
                                start=True, stop=True,
                            )
                            nc.tensor.matmul(
                                ps_b[64:128, 0:nch],
                                wc_sb[:],
                                csb[:, 0:nch],
                                start=True,
                                stop=True,
                                tile_position=(0, 64),
                            )
                            x0a = p0.tile(
                                [128, NCH], f16, tag="x0a", name="x0a"
                            )
                            lrelu_from(
                                p0, x0a[:, 0:nch], ps_a[:, 0:nch], ba_sb[:],
                                [128, nch],
                            )
                            x0b = p0.tile(
                                [128, NCH], f16, tag="x0b", name="x0b"
                            )
                            lrelu_from(
                                p0, x0b[:, 0:nch], ps_b[:, 0:nch], bb_sb[:],
                                [128, nch],
                            )

                            for h, xdst in ((0, x1a[q]), (1, x1b[q])):
                                ps_x = p0ps.tile(
                                    [128, NCH], f32, tag="psx", name="ps_x"
                                )
                                nc.tensor.matmul(
                                    ps_x[:, 0:nch],
                                    wi_sb[:, 0, h * 128 : (h + 1) * 128],
                                    x0a[:, 0:nch],
                                    start=True,
                                    stop=False,
                                )
                                nc.tensor.matmul(
                                    ps_x[:, 0:nch],
                                    wi_sb[:, 1, h * 128 : (h + 1) * 128],
                                    x0b[:, 0:nch],
                                    start=False,
                                    stop=True,
                                )
                                lrelu_from(
                                    p0,
                                    xdst[:, c0 : c0 + nch],
                                    ps_x[:, 0:nch],
                                    bi_sb[:, h : h + 1],
                                    [128, nch],
                                )
                            c0 += nch
                        produce_h_quarter(
                            q, x1a[q], x1b[q], w1_sb, h1_loc, p0, p0ps
                        )
                        AG(h1_loc[q], h1_full[q])

                # -------- RGCN layer: quarter-outer aggregation --------
                def rgcn_layer(
                    xa_at, xb_at, r_sb, b_sb, h_full_q, out_cb, lp, lps
                ):
                    # root + bias -> acc (runs during the AllGathers)
                    for t in range(T):
                        rpo = lps.tile([128, 256], f32, tag="paux", name="rpo")
                        nc.tensor.matmul(
                            rpo[:], xa_at(t), r_sb[:, 0, :],
                            start=True, stop=False,
                        )
                        nc.tensor.matmul(
                            rpo[:], xb_at(t), r_sb[:, 1, :],
                            start=False, stop=True,
                        )
                        nc.vector.tensor_add(acc[:, t, :], rpo[:], b_sb[:])
                    # aggregation, quarter-outer so AG(q+1) overlaps work(q)
                    for g in range(NQUART):
                        rows_q = m.q_tab_rows[g]
                        for t in range(T):
                            B = int(m.bud[t, g])
                            if B == 0:
                                continue
                            nbg = int(m.gnb[t, g])
                            nboff = int(m.nboff[t, g])
                            o = int(m.goff[t, g])
                            qn = t % nq
                            ssb = lp.tile(
                                [128, gnb_max, 128], f16,
                                tag="ssb", name="ssb", bufs=3,
                            )
                            nc.sync.dma_start(
                                ssb[:, 0:nbg, :],
                                sblk_ext.ap()[:, nboff : nboff + nbg, :],
                            )
                            msg = lp.tile(
                                [128, gnb_max, 256], f16,
                                tag=f"msg{qn}", name=f"msg{qn}", bufs=4,
                            )
                            nc.gpsimd.dma_gather(
                                msg[:, 0:nbg, :],
                                h_full_q[g].ap()[0:rows_q, :],
                                idx_sb[:, o // 16 : (o + B) // 16],
                                num_idxs=B,
                                num_idxs_reg=B,
                                elem_size=256,
                                single_packet="sp0" not in variant,
                                queue_num=qn,
                            )
                            po = lps.tile(
                                [128, 256], f32, tag="po", name="po", bufs=4
                            )
                            for b in range(nbg):
                                K = min(128, B - b * 128)
                                nc.tensor.matmul(
                                    po[:],
                                    ssb[0:K, b, :],
                                    msg[0:K, b, :],
                                    start=(b == 0),
                                    stop=(b == nbg - 1),
                                )
                            nc.vector.tensor_add(
                                acc[:, t, :], acc[:, t, :], po[:]
                            )
                    # emit outputs
                    for t in range(T):
                        ts = slice(t * 128, (t + 1) * 128)
                        osb = lp.tile([128, 256], f16, tag="osb", name="osb")
                        nc.vector.tensor_copy(osb[:], acc[:, t, :])
                        out_cb(t, ts, osb, lp, lps)

                h2_stage = [None, None]

                def l1_out(t, ts, osb, lp, lps):
                    for h, xdst in ((0, x2a), (1, x2b)):
                        pt = lps.tile([128, 128], f16, tag="pt", name="pt")
                        nc.tensor.transpose(
                            pt[:], osb[:, h * 128 : (h + 1) * 128], id_sb[:]
                        )
                        nc.vector.tensor_copy(xdst[:, ts], pt[:])
                    j = t % 4
                    if j == 0:
                        h2_stage[0] = lp.tile(
                            [128, 4, 256], f16, tag="h2s0", name="h2s0"
                        )
                        h2_stage[1] = lp.tile(
                            [128, 4, 256], f16, tag="h2s1", name="h2s1"
                        )
                    for r in range(2):
                        ph = lps.tile([128, 256], f32, tag="paux", name="ph2")
                        nc.tensor.matmul(
                            ph[:], x2a[:, ts], w2_sb[:, 2 * r, :],
                            start=True, stop=False,
                        )
                        nc.tensor.matmul(
                            ph[:], x2b[:, ts], w2_sb[:, 2 * r + 1, :],
                            start=False, stop=True,
                        )
                        nc.vector.tensor_copy(h2_stage[r][:, j, :], ph[:])
                    # flush h2 stage at quarter boundaries (or every 4 tiles)
                    qq = int(np.searchsorted(m.q_start, t, "right")) - 1
                    qt0 = int(m.q_start[qq])
                    qt1 = int(m.q_start[qq + 1])
                    if j == 3 or t == qt1 - 1:
                        t0 = t - j
                        nt = j + 1
                        # may straddle a quarter boundary: split per quarter
                        tcur = t0
                        while tcur <= t:
                            q2 = int(np.searchsorted(m.q_start, tcur, "right")) - 1
                            q2end = int(m.q_start[q2 + 1])
                            tstop = min(t, q2end - 1)
                            cnt_t = tstop - tcur + 1
                            for r in range(2):
                                row0 = (
                                    r * m.q_rows[q2]
                                    + (tcur - int(m.q_start[q2])) * 128
                                )
                                nc.sync.dma_start(
                                    h2_loc[q2]
                                    .ap()[row0 : row0 + cnt_t * 128, :]
                                    .rearrange("(b p) f -> p b f", p=128),
                                    h2_stage[r][:, tcur - t0 : tcur - t0 + cnt_t, :],
                                )
                            tcur = tstop + 1

                def l2_out(t, ts, osb, lp, lps):
                    o2t = lp.tile([128, 2, 128], f16, tag="o2t", name="o2t")
                    for h in range(2):
                        pt = lps.tile([128, 128], f16, tag="pt", name="pt")
                        nc.tensor.transpose(
                            pt[:], osb[:, h * 128 : (h + 1) * 128], id_sb[:]
                        )
                        nc.vector.tensor_copy(o2t[:, h, :], pt[:])
                    ht = lp.tile([128, 2, 128], f16, tag="ht", name="ht")
                    for h in range(2):
                        phd = lps.tile([128, 256], f32, tag="paux", name="phd")
                        nc.tensor.matmul(
                            phd[:, 0:128],
                            wo1_sb[:, 0, h * 128 : (h + 1) * 128],
                            o2t[:, 0, :],
                            start=True,
                            stop=False,
                        )
                        nc.tensor.matmul(
                            phd[:, 0:128],
                            wo1_sb[:, 1, h * 128 : (h + 1) * 128],
                            o2t[:, 1, :],
                            start=False,
                            stop=True,
                        )
                        lrelu_from(
                            lp, ht[:, h, :], phd[:, 0:128], bo1_sb[:, h : h + 1],
                            [128, 128],
                        )
                    pf = lps.tile([128, 256], f32, tag="paux", name="pf")
                    nc.tensor.matmul(
                        pf[:, 0:2], ht[:, 0, :], wo2_sb[:, 0, :],
                        start=True, stop=False,
                    )
                    nc.tensor.matmul(
                        pf[:, 0:2], ht[:, 1, :], wo2_sb[:, 1, :],
                        start=False, stop=True,
                    )
                    nc.vector.tensor_add(
                        out_stage[:, t, :], pf[:, 0:2], bo2_sb[:]
                    )
                    if t == T - 1:
                        nc.sync.dma_start(
                            out_ext.ap().rearrange("(b p) f -> p b f", p=128),
                            out_stage[:],
                        )

                with (
                    tc.tile_pool(name="lyr", bufs=2) as lp,
                    tc.tile_pool(name="lyrps", bufs=2, space="PSUM") as lps,
                ):
                    q_of = [0] * T
                    for q in range(NQUART):
                        for t in range(int(m.q_start[q]), int(m.q_start[q + 1])):
                            q_of[t] = q

                    def x1a_at(t):
                        q = q_of[t]
                        lt = t - int(m.q_start[q])
                        return x1a[q][:, lt * 128 : (lt + 1) * 128]

                    def x1b_at(t):
                        q = q_of[t]
                        lt = t - int(m.q_start[q])
                        return x1b[q][:, lt * 128 : (lt + 1) * 128]

                    rgcn_layer(
                        x1a_at, x1b_at, r1_sb, b1_sb, h1_full, l1_out, lp, lps
                    )
                    for q in range(NQUART):
                        AG(h2_loc[q], h2_full[q])
                    rgcn_layer(
                        lambda t: x2a[:, t * 128 : (t + 1) * 128],
                        lambda t: x2b[:, t * 128 : (t + 1) * 128],
                        r2_sb, b2_sb, h2_full, l2_out, lp, lps,
                    )

            for _rep in range(repeats):
                whole_body()

    nc.compile()
    return nc


def _pmaj(t):
    """DRAM tensor [a, 128, b] viewed partition-major [128, a, b]."""
    return t.ap().rearrange("a p b -> p a b")


# ---------------------------------------------------------------- entry
def kernel(**inputs):
    meta, in_maps = prepare(inputs)
    nc = build(meta, variant=DEFAULT_VARIANT)
    from concourse.bass_utils import run_bass_kernel_spmd

    res = run_bass_kernel_spmd(
        nc, in_maps, core_ids=list(range(meta.ncores))
    ).results
    out = np.concatenate(
        [res[c]["out"][: meta.nsh] for c in range(meta.ncores)], axis=0
    )
    return out.astype(np.float32)
